# revision 33
# baseline (speedup 1.0000x reference)
"""Causal self-attention (B=2, T=2048, D=1024, H=16, Dh=64) on 8 NeuronCores.

Sharding: tensor-parallel over heads. Core c owns heads {2c, 2c+1}:
  - QKV: computes q/k/v columns c*128:(c+1)*128 of each section.
      q,k are produced transposed (qT/kT: [128 qkv-cols, tokens]) via
      out = w3_slice.T @ x.T matmuls; v is produced in natural layout
      ([tokens, 128 v-cols]) via PE transposes of the vT chunks.
  - Attention: for each (batch, q-chunk of 512 queries, k-tile of 128):
      S^T = K_h @ Q_h.T from kT/qT, exp on ACT, causal mask via in-place
      affine_select on the 128-col diagonal band only, then out^T
      accumulated as V'.T @ P^T where V' = [V | ones]: row 64 of the PSUM
      accumulator is the softmax denominator.
  - Projection: partial out^T = w_proj_slice.T applied per 128-row slice;
      per-core partial [1024, 4096] outputs are summed on the host.

v2 scheduling (what changed vs the 233 us baseline):
  - Clock-model-driven interleaved emission: the per-engine queues are
    in-order, so a coarse phase order (qkv0, attn0, qkv1, attn1) left the
    PE starved during attn's exp-bound stretches (HAM clock-gated to
    1.2 GHz for ~38 us) while qkv(1)/proj work sat behind it in program
    order. Emission now tracks estimated per-engine clocks and pumps
    filler quanta (qkv nt-groups, proj ob-slices) into predicted PE
    bubbles, so the PE streams continuously at 2.4 GHz.
  - Triangular truncation: on diagonal k-tiles only columns >= c0 (the
    un-masked range) are computed by S/exp/PV, and the causal mask is an
    in-place affine_select on just the [128,128] boundary band (base=0
    uniform across tiles) instead of a full [128,512] copy per head.
  - Softmax normalize without the DRAM bounce: denominator rows are cast
    to bf16, fanned out across partitions with a tiny ones-matmul
    (K=2, both heads at once), reciprocal'd as a [128,512] DVE op and
    multiplied into aT with one GPSIMD tensor_tensor. bf16 rounding of
    the denominator is a pure per-query scale (<2e-3, invisible to the
    relative-error metric).
  - attn(1) q-chunk order 1,2,3,0: the tail chunk has only 4 k-tiles, so
    the serial end-of-kernel chain (attn + normalize + proj + out DMA)
    is minimal.
  - x chunk 0 is DMA'd in two 256-token halves and QKV mc0 runs as
    256-moving accumulation groups so the PE starts ~3 us earlier.
  - proj drains alternate GPSIMD/DVE and each [128,512] ob slice DMAs
    out individually (2 KB/partition descriptors) for a smooth tail.

All matmuls run in float32r (4-byte data, reduced-precision multiply,
~1 cycle/row measured at 512-moving warm).
"""

from collections import deque

import numpy as np

D_MODEL = 1024
B, T = 2, 2048
RC = 128  # per-core qkv columns per q/k/v section == per-core w_proj rows
M = B * T
N_CORES = 8

_prog_cache = {}
_last_results = None  # BassKernelResults of the most recent run (for profiling)


def build_program(Tb=T, use_vbias=False):
    from contextlib import ExitStack

    import concourse.bass as bass
    import concourse.tile as tile
    from concourse import bacc, mybir
    from concourse.tile import add_dep_helper

    f32 = mybir.dt.float32
    f32r = mybir.dt.float32r
    bf16 = mybir.dt.bfloat16
    EXP = mybir.ActivationFunctionType.Exp
    MULT = mybir.AluOpType.mult
    IS_GE = mybir.AluOpType.is_ge
    IS_EQ = mybir.AluOpType.is_equal

    mc_per_b = Tb // 512  # x/m chunks of 512 tokens per batch
    mt_per_b = Tb // 128  # v tiles of 128 tokens per batch
    n_qc = Tb // 512      # query chunks per batch

    nc = bacc.Bacc("TRN2", target_bir_lowering=False, debug=False)
    xq = nc.dram_tensor("xq", [B * Tb // 512, 128, 8, 512], f32r,
                        kind="ExternalInput").ap()
    w3 = nc.dram_tensor("w3", [128, 8, 3 * RC], f32r, kind="ExternalInput").ap()
    wp = nc.dram_tensor("wp", [RC, D_MODEL], f32r, kind="ExternalInput").ap()
    bqk = nc.dram_tensor("bqk", [RC, 2], f32, kind="ExternalInput").ap()
    bv = None
    if use_vbias:
        bv = nc.dram_tensor("bv", [RC, 1], f32, kind="ExternalInput").ap()
    out_d = nc.dram_tensor("out", [D_MODEL, B * Tb], f32, kind="ExternalOutput").ap()

    out_r = out_d.rearrange("(nt p) m -> p nt m", p=128)  # [128, 8, Mb]

    # --- emission clock model (ns); only shapes queue order, sync is real.
    # pe starts at ~the real time the first QKV matmul can issue (x0 half0
    # landed) so the x_arrival gates compare on the same scale.
    CLK = {"pe": 9000.0, "act": 9000.0, "gps": 9000.0, "dve": 9000.0}
    import os as _os
    sched_log = [] if _os.environ.get("SCHED_DEBUG") else None

    def slog(kind):
        if sched_log is not None:
            sched_log.append((kind, CLK["pe"], CLK["act"], CLK["gps"]))

    def mm_ns(mov):
        # f32r: 1 cyc/row at >=256 moving, 4 cyc/row below; ~20ns issue
        return max(213.0, mov * 0.417) + 20.0

    def exp_ns(cols):
        return 2 * 128 * cols / 117.8 + 60.0

    with tile.TileContext(nc) as tc:
        with ExitStack() as ctx:
            singles = ctx.enter_context(tc.tile_pool(name="singles", bufs=1))
            xpool = ctx.enter_context(tc.tile_pool(name="xpool", bufs=3))
            ptp = ctx.enter_context(tc.tile_pool(name="ptp", bufs=3))
            vtp = ctx.enter_context(tc.tile_pool(name="vtp", bufs=2))
            pvcp = ctx.enter_context(tc.tile_pool(name="pvcp", bufs=2))
            rbbp = ctx.enter_context(tc.tile_pool(name="rbbp", bufs=2))
            obp = ctx.enter_context(tc.tile_pool(name="obp", bufs=8))
            ps_a = ctx.enter_context(tc.tile_pool(name="ps_a", bufs=2, space="PSUM"))
            ps_s = ctx.enter_context(tc.tile_pool(name="ps_s", bufs=3, space="PSUM"))
            ps_p = ctx.enter_context(tc.tile_pool(name="ps_p", bufs=1, space="PSUM"))
            ps_pv = ctx.enter_context(tc.tile_pool(name="ps_pv", bufs=2, space="PSUM"))

            # identity built on-chip (memset ones + diagonal affine_select)
            # so PE warmup starts immediately instead of waiting a DMA;
            # warmup matmuls release the HAM clock gate by the time the
            # first x piece lands
            id_sb = singles.tile([128, 128], f32r, tag="ident")
            nc.vector.memset(id_sb.bitcast(f32), 1.0)
            nc.gpsimd.affine_select(
                id_sb, id_sb, pattern=[[1, 128]], compare_op=IS_EQ,
                fill=0.0, base=0, channel_multiplier=-1,
            )
            # ones33/dn33: exact f32 denominator broadcast via one full-fp32
            # matmul. The two denominator rows live on partitions 0 and 32
            # (legal partition starts); rows 1..31 stay zero from the init
            # memset so they contribute nothing to the K=33 contraction.
            ones33 = singles.tile([33, 128], f32, tag="on33")
            dn33 = singles.tile([33, 512], f32, tag="dn33")
            nc.vector.memset(ones33, 0.0)
            nc.vector.memset(dn33, 0.0)
            nc.vector.memset(ones33[0:1, 0:64], 1.0)
            nc.vector.memset(ones33[32:33, 64:128], 1.0)
            wu_ps = ps_a.tile([128, 512], f32, tag="mm")
            for _ in range(28):
                nc.tensor.matmul(wu_ps[:, 0:128], id_sb, id_sb,
                                 start=True, stop=True)

            def warm(n):
                # dummy matmuls sprinkled through the latency-bound tail:
                # they hold the HAM clock gate at 8/8 so the sparse real
                # matmuls there run at 2.4 GHz instead of 1.2
                ds = ps_s.tile([128, 512], f32, tag="s")
                for _ in range(n):
                    nc.tensor.matmul(ds[:, 0:128], id_sb, id_sb,
                                     start=True, stop=True)

            # x chunks prefetched on the SP HWDGE ring; weights go through
            # the ACT HWDGE ring so the two streams don't serialize.
            # Chunk 0 is split into four kt-pair pieces (contiguous 4 KiB
            # runs per partition) chained in order: the first QKV matmuls
            # only need their own kt slice, so accumulation starts as soon
            # as piece 0 lands instead of after the whole chunk.
            x_tiles, x_dmas = [], []
            x_arrival = []
            for mc in range(B * mc_per_b):
                x_sb = xpool.tile([128, 8, 512], f32r, tag="x")
                if mc == 0:
                    prev = None
                    for p in range(4):
                        xd = nc.sync.dma_start(x_sb[:, 2 * p:2 * p + 2, :],
                                               xq[0][:, 2 * p:2 * p + 2, :])
                        if prev is not None:
                            add_dep_helper(xd.ins, prev.ins,
                                           reason="x0 piece pacing")
                        prev = xd
                elif mc == 1:
                    xd = nc.sync.dma_start(x_sb, xq[mc])
                    # chunk 0 gets the full HBM bandwidth: QKV can't start
                    # without it, so its arrival time is the kernel prologue
                    add_dep_helper(xd.ins, x_dmas[0].ins,
                                   reason="x DMA pacing")
                else:
                    xd = nc.sync.dma_start(x_sb, xq[mc])
                    add_dep_helper(xd.ins, x_dmas[mc - 2].ins,
                                   reason="x DMA pacing")
                x_tiles.append(x_sb)
                x_dmas.append(xd)
                x_arrival.append(11000.0 + 4300.0 * mc)

            w3_sb = singles.tile([128, 8, 3 * RC], f32r, tag="w3")
            nc.scalar.dma_start(w3_sb, w3)
            bqk_sb = singles.tile([RC, 2], f32, tag="bqk")
            nc.scalar.dma_start(bqk_sb, bqk)
            wp_sb = singles.tile([128, D_MODEL], f32r, tag="wp")
            nc.scalar.dma_start(wp_sb, wp)
            bv_sb = None
            if use_vbias:
                bv_sb = singles.tile([RC, 1], f32, tag="bv")
                nc.scalar.dma_start(bv_sb, bv)

            qT, kT, vb, aT = {}, {}, {}, {}
            for b in range(B):
                qT[b] = singles.tile([128, Tb], f32r, tag=f"qT{b}", name=f"qT{b}")
                kT[b] = singles.tile([128, Tb], f32r, tag=f"kT{b}", name=f"kT{b}")
                vb[b] = singles.tile([128, mt_per_b, 130], f32r, tag=f"vb{b}",
                                     name=f"vb{b}")
                aT[b] = singles.tile([128, Tb], f32r, tag=f"aT{b}", name=f"aT{b}")
                # ones columns for the softmax-denominator rows of PV
                nc.vector.memset(vb[b][:, :, 64:65].bitcast(f32), 1.0)
                nc.vector.memset(vb[b][:, :, 129:130].bitcast(f32), 1.0)

            # ---------------- quantum emitters ----------------

            def emit_qkv_group(b, mci, nt, mlo, mhi):
                """One accumulation group: qkv columns of section nt for
                tokens [mlo, mhi) of chunk mci (plus its drain op)."""
                mc = b * mc_per_b + mci
                x_sb = x_tiles[mc]
                mv = mhi - mlo
                ps = ps_a.tile([128, mv], f32, tag="mm")
                for kt in range(8):
                    nc.tensor.matmul(
                        ps,
                        w3_sb[:, kt, nt * RC:(nt + 1) * RC],
                        x_sb[:, kt, mlo:mhi],
                        start=(kt == 0), stop=(kt == 7),
                    )
                CLK["pe"] += 8 * mm_ns(mv)
                if nt < 2:
                    dest = qT[b] if nt == 0 else kT[b]
                    nc.vector.tensor_scalar_add(
                        dest[:, mci * 512 + mlo:mci * 512 + mhi], ps,
                        bqk_sb[:, nt:nt + 1],
                    )
                    CLK["dve"] += 750.0 * mv / 512
                slog(f"qkv{b}m{mci}n{nt}")
                if False:
                    pass
                else:
                    vTs = qkv_state[b]["vTs"]
                    if vTs is None:
                        vTs = vtp.tile([128, 512], f32r, tag="vT")
                        qkv_state[b]["vTs"] = vTs
                    nc.vector.tensor_copy(vTs[:, mlo:mhi], ps)
                    CLK["dve"] += 690.0 * mv / 512

            def emit_qkv_vtrans(b, mci):
                """Transpose vT chunks into natural [tokens, vcol] layout."""
                vTs = qkv_state[b]["vTs"]
                qkv_state[b]["vTs"] = None
                tp = ps_a.tile([128, 512], f32, tag="mm")
                for ms in range(4):
                    nc.tensor.transpose(
                        tp[:, ms * 128:(ms + 1) * 128].bitcast(f32r),
                        vTs[:, ms * 128:(ms + 1) * 128],
                        id_sb,
                    )
                CLK["pe"] += 4 * 160.0
                for ms in range(4):
                    mt = mci * 4 + ms
                    sl = tp[:, ms * 128:(ms + 1) * 128].bitcast(f32r)
                    # single strided copy covering V cols {0:64} u {65:129}
                    dst = vb[b][:, mt, :].rearrange("p (two f) -> p two f",
                                                    f=65)[:, :, 0:64]
                    src = sl.rearrange("p (two f) -> p two f", f=64)
                    nc.vector.tensor_copy(dst, src)
                    CLK["dve"] += 340.0
                slog(f"qkv{b}m{mci}T")

            qkv_state = {b: {"vTs": None} for b in range(B)}

            proj_pending = deque()  # (avail_ns, b, q_lo, qw, np_)
            ob_alt = [0]

            def emit_proj_np(b, q_lo, qw, np_):
                for j in range(2):
                    nt = np_ * 2 + j
                    ps_t = ps_p.tile([128, 512], f32, tag="pp")
                    ps = ps_t[:, 0:qw]
                    nc.tensor.matmul(
                        ps,
                        wp_sb[:, nt * 128:(nt + 1) * 128],
                        aT[b][:, q_lo:q_lo + qw],
                        start=True, stop=True,
                    )
                    CLK["pe"] += mm_ns(qw)
                    ob_t = obp.tile([128, 512], f32, tag="ob")
                    ob = ob_t[:, 0:qw]
                    # GPSIMD cannot read PSUM; alternate the drain between
                    # the ACT and DVE engines to balance their queues
                    if ob_alt[0] % 2 == 0:
                        nc.scalar.copy(ob, ps)
                        CLK["act"] += 580.0 * qw / 512
                    else:
                        nc.vector.tensor_copy(ob, ps)
                        CLK["dve"] += 690.0 * qw / 512
                    ob_alt[0] += 1
                    slog(f"proj{b}q{q_lo // 512}n{np_}")
                    nc.sync.dma_start(
                        out_r[:, nt, b * Tb + q_lo: b * Tb + q_lo + qw],
                        ob,
                    )

            class QkvStream:
                def __init__(self, b):
                    self.b = b
                    self.items = []
                    for mci in range(mc_per_b):
                        mc = b * mc_per_b + mci
                        for nt in range(3):
                            self.items.append(
                                ("g", mci, nt, 0, 512, x_arrival[mc]))
                        self.items.append(("t", mci, 0, 0, 0, x_arrival[mc]))
                    self.pos = 0

                def done_through(self, mci):
                    """True if all quanta of chunks <= mci are emitted."""
                    for it in self.items[self.pos:]:
                        if it[1] <= mci:
                            return False
                    return True

                def avail(self):
                    if self.pos >= len(self.items):
                        return None
                    return self.items[self.pos][5]

                def step(self):
                    it = self.items[self.pos]
                    self.pos += 1
                    if it[0] == "g":
                        emit_qkv_group(self.b, it[1], it[2], it[3], it[4])
                    else:
                        emit_qkv_vtrans(self.b, it[1])

            qkv_streams = {b: QkvStream(b) for b in range(B)}

            pump_alt = [0]

            def _pump_qkv():
                for b in range(B):
                    qs = qkv_streams[b]
                    av = qs.avail()
                    if av is not None and av <= CLK["pe"] + 1500.0:
                        qs.step()
                        return True
                return False

            def _pump_proj():
                if not proj_pending:
                    return False
                avail, b, q_lo, qw, np_ = proj_pending[0]
                if avail > CLK["pe"] + 800.0:
                    return False
                proj_pending.popleft()
                emit_proj_np(b, q_lo, qw, np_)
                return True

            def pump(target):
                """Emit filler quanta until the estimated PE clock reaches
                target (or nothing is ready). Round-robin qkv/proj so the
                output DMA stream flows throughout instead of piling up at
                the tail."""
                while CLK["pe"] < target:
                    pump_alt[0] ^= 1
                    first, second = ((_pump_qkv, _pump_proj)
                                     if pump_alt[0] else
                                     (_pump_proj, _pump_qkv))
                    if not (first() or second()):
                        break

            def ensure_qkv(b, mci):
                qs = qkv_streams[b]
                while not qs.done_through(mci):
                    qs.step()

            # ---------------- attention ----------------

            def emit_attn_chunk(b, q_lo, qw, next_need=None, proj_keep=3,
                                tail_warm=False):
                """Attention for queries [q_lo, q_lo+qw) of batch b.
                next_need=(b', mci'): the qkv chunk the NEXT attention
                chunk requires; its quanta are spread across this chunk's
                k-tiles instead of bulk-emitted at the boundary. proj_keep:
                force-drain proj backlog beyond this depth (1 quantum/kt)
                so the out-DMA stream flows instead of piling at the tail."""
                nkt = (q_lo + qw) // 128
                kt_diag = q_lo // 128
                pv0_t = ps_pv.tile([65, 512], f32, tag="pv", name="pv0")
                pv1_t = ps_pv.tile([65, 512], f32, tag="pv", name="pv1")
                pvs = (pv0_t[:, 0:qw], pv1_t[:, 0:qw])
                pend = None  # (kt, c0, psrc, p_done_est)

                def spread(kt):
                    if next_need is not None:
                        qs = qkv_streams[next_need[0]]
                        if (not qs.done_through(next_need[1])
                                and kt % 3 == 2):
                            qs.step()
                            return
                    if len(proj_pending) > proj_keep:
                        avail, pb, pq, pw, pnp = proj_pending[0]
                        if avail <= CLK["pe"] + 800.0:
                            proj_pending.popleft()
                            emit_proj_np(pb, pq, pw, pnp)

                def emit_pv(kt, c0, psrc, p_done):
                    pump(p_done - 500.0)
                    for h in (0, 1):
                        nc.tensor.matmul(
                            pvs[h][:, c0:qw],
                            vb[b][:, kt, h * 65:(h + 1) * 65],
                            psrc[:, h, c0:qw],
                            start=(kt == 0), stop=(kt == nkt - 1),
                        )
                    CLK["pe"] = max(CLK["pe"], p_done) + 2 * mm_ns(qw - c0)
                    slog(f"pv{b}q{q_lo // 512}k{kt}")
                    if tail_warm and kt >= nkt - 6:
                        warm(2)

                for kt in range(nkt):
                    c0 = max(0, kt * 128 - q_lo)
                    diag = kt >= kt_diag
                    pt_t = ptp.tile([128, 2, 512], f32r, tag="pt")
                    pt = pt_t[:, :, 0:qw]
                    # per-head S tiles (1 PSUM bank each) and per-head exp:
                    # head 0's mask runs on GPSIMD while head 1's exp is
                    # still on ACT, and the freed bank gives proj a
                    # dedicated psum pool (no PE-queue head-of-line blocking
                    # on psum when proj filler is pumped mid-attention)
                    for h in (0, 1):
                        sh_t = ps_s.tile([128, 512], f32, tag="s")
                        sh = sh_t[:, 0:qw]
                        nc.tensor.matmul(
                            sh[:, c0:qw],
                            kT[b][h * 64:(h + 1) * 64,
                                  kt * 128:(kt + 1) * 128],
                            qT[b][h * 64:(h + 1) * 64,
                                  q_lo + c0:q_lo + qw],
                            start=True, stop=True,
                        )
                        CLK["pe"] += mm_ns(qw - c0)
                        nc.scalar.activation(pt[:, h, c0:qw], sh[:, c0:qw],
                                             EXP, scale=0.125)
                        CLK["act"] = max(CLK["act"], CLK["pe"] + 150.0) \
                            + exp_ns(qw - c0) / 2
                        if diag:
                            nc.gpsimd.affine_select(
                                pt[:, h, c0:c0 + 128],
                                pt[:, h, c0:c0 + 128],
                                pattern=[[1, 128]],
                                compare_op=IS_GE,
                                fill=0.0,
                                base=0,
                                channel_multiplier=-1,
                            )
                    p_done = CLK["act"]
                    if diag:
                        CLK["gps"] = max(CLK["gps"], p_done) + 250.0
                        p_done = CLK["gps"]
                    if pend is not None:
                        emit_pv(*pend)
                        spread(kt)
                    pend = (kt, c0, pt, p_done)
                emit_pv(*pend)

                # normalize: denom rows -> f32 ones-matmul partition
                # broadcast -> fast reciprocal -> one fused multiply into aT
                d0 = q_lo % 512
                pvc_t = pvcp.tile([128, 512], f32, tag="pvc")
                pvc = pvc_t[:, 0:qw]
                nc.vector.tensor_copy(pvc[0:64], pvs[0][0:64])
                nc.vector.tensor_copy(pvc[64:128], pvs[1][0:64])
                nc.vector.tensor_copy(dn33[0:1, d0:d0 + qw], pvs[0][64:65])
                nc.vector.tensor_copy(dn33[32:33, d0:d0 + qw], pvs[1][64:65])
                CLK["dve"] = max(CLK["dve"], CLK["pe"]) \
                    + (2 * 350.0 + 2 * 270.0) * qw / 512
                pump(CLK["dve"] - 200.0)  # bcast matmul waits the dn copies
                ps_rb_t = ps_a.tile([128, 512], f32, tag="mm")
                ps_rb = ps_rb_t[:, 0:qw]
                nc.tensor.matmul(ps_rb, ones33, dn33[:, d0:d0 + qw],
                                 start=True, stop=True)
                CLK["pe"] = max(CLK["pe"], CLK["dve"]) + qw * 1.67 + 50.0
                rbb_t = rbbp.tile([128, 512], f32, tag="rbb")
                rbb = rbb_t[:, 0:qw]
                nc.vector.reciprocal_approx_fast(rbb, ps_rb)
                CLK["dve"] = max(CLK["dve"], CLK["pe"]) + 700.0 * qw / 512
                dst = aT[b][:, q_lo:q_lo + qw]
                nc.gpsimd.tensor_tensor(dst, pvc, rbb, op=MULT)
                CLK["gps"] = max(CLK["gps"], CLK["dve"]) + 700.0 * qw / 512
                slog(f"norm{b}q{q_lo // 512}")
                if tail_warm:
                    warm(3)
                if use_vbias:
                    nc.vector.tensor_scalar_add(dst, dst, bv_sb)
                avail = CLK["gps"]
                for np_ in range(4):
                    proj_pending.append((avail, b, q_lo, qw, np_))

            # ---------------- top-level schedule ----------------

            # attention chunk order: b0 ascending, then b1 chunks 1,2,3
            # and finally b1 queries [256,512) then [0,256) — the very last
            # chunk has only 2 k-tiles so the serial end-of-kernel chain
            # (attn + normalize + proj + out DMA) is minimal. next_need
            # pre-spreads the qkv chunk the following attention chunk needs.
            order = [(0, 0, 512), (0, 512, 512), (0, 1024, 512),
                     (0, 1536, 512),
                     (1, 512, 512), (1, 1024, 512), (1, 1536, 512),
                     (1, 0, 512)]
            for i, (b, q_lo, qw) in enumerate(order):
                ensure_qkv(b, (q_lo + qw - 1) // 512)
                if i + 1 < len(order):
                    nb, nq, nw = order[i + 1]
                    nxt = (nb, (nq + nw - 1) // 512)
                else:
                    nxt = None
                tail = i >= len(order) - 2
                emit_attn_chunk(b, q_lo, qw, next_need=nxt,
                                proj_keep=0 if tail else 3, tail_warm=tail)
            # drain leftovers
            ensure_qkv(1, mc_per_b - 1)
            while proj_pending:
                _, b, q_lo, qw, np_ = proj_pending.popleft()
                emit_proj_np(b, q_lo, qw, np_)
                warm(1)

    if sched_log is not None:
        print("=== emission schedule (kind, pe_us) ===")
        line = []
        for kind, pe, act, gps in sched_log:
            line.append(f"{kind}@{pe/1000:.0f}")
            if len(line) == 10:
                print("  " + " ".join(line)); line = []
        if line:
            print("  " + " ".join(line))
        print(f"final CLK: pe={CLK['pe']/1000:.1f} act={CLK['act']/1000:.1f} "
              f"gps={CLK['gps']/1000:.1f} dve={CLK['dve']/1000:.1f}")
    nc.compile()
    return nc


def make_in_maps(x, w_qkv, b_qkv, use_vbias):
    """Host-side shard prep. Returns per-core input maps (w_proj added later)."""
    Mx = x.shape[0] * x.shape[1]
    # [chunks, 128p, 8kt, 512m]: per-partition-contiguous 16 KiB blocks so
    # each chunk DMA uses 128 big descriptors instead of 1024 2-KiB ones
    xq = np.ascontiguousarray(
        x.reshape(Mx // 512, 512, 8, 128).transpose(0, 3, 2, 1)
    )
    in_maps = []
    for c in range(N_CORES):
        w3c = np.concatenate(
            [w_qkv[:, s * D_MODEL + c * RC: s * D_MODEL + (c + 1) * RC]
             for s in range(3)],
            axis=1,
        )
        # [128p, 8kt, 384]: per-partition-contiguous blocks for big DMA
        # descriptors
        w3c = np.ascontiguousarray(
            w3c.reshape(8, 128, 3 * RC).transpose(1, 0, 2)
        )
        bqkc = np.ascontiguousarray(
            np.stack(
                [b_qkv[c * RC:(c + 1) * RC],
                 b_qkv[D_MODEL + c * RC: D_MODEL + (c + 1) * RC]],
                axis=1,
            )
        )
        im = {"xq": xq, "w3": w3c, "bqk": bqkc,
              "ident": np.eye(128, dtype=np.float32)}
        if use_vbias:
            im["bv"] = np.ascontiguousarray(
                b_qkv[2 * D_MODEL + c * RC: 2 * D_MODEL + (c + 1) * RC][:, None]
            )
        in_maps.append(im)
    return in_maps


def kernel(x, w_qkv, b_qkv, w_proj, b_proj):
    from concourse.bass_utils import run_bass_kernel_spmd

    x = np.asarray(x, dtype=np.float32)
    w_qkv = np.asarray(w_qkv, dtype=np.float32)
    b_qkv = np.asarray(b_qkv, dtype=np.float32)
    w_proj = np.asarray(w_proj, dtype=np.float32)
    b_proj = np.asarray(b_proj, dtype=np.float32)

    use_vbias = bool(np.any(b_qkv[2 * D_MODEL:]))
    key = (T, use_vbias)
    if key not in _prog_cache:
        _prog_cache[key] = build_program(T, use_vbias)
    nc = _prog_cache[key]

    in_maps = make_in_maps(x, w_qkv, b_qkv, use_vbias)
    for c in range(N_CORES):
        in_maps[c]["wp"] = np.ascontiguousarray(w_proj[c * RC:(c + 1) * RC, :])

    res = run_bass_kernel_spmd(nc, in_maps, core_ids=list(range(N_CORES)))
    global _last_results
    _last_results = res
    total = res.results[0]["out"].copy()
    for c in range(1, N_CORES):
        total += res.results[c]["out"]
    out = total.T.reshape(B, T, D_MODEL) + b_proj[None, None, :]
    return np.ascontiguousarray(out.astype(np.float32))


# revision 35
# speedup vs baseline: 1.1561x; 1.1561x over previous
"""Causal self-attention (B=2, T=2048, D=1024, H=16, Dh=64) on 8 NeuronCores.

Sharding: tensor-parallel over heads. Core c owns heads {2c, 2c+1}:
  - QKV: computes q/k/v columns c*128:(c+1)*128 of each section.
      q,k are produced transposed (qT/kT: [128 qkv-cols, tokens]) via
      out = w3_slice.T @ x.T matmuls; v is produced in natural layout
      ([tokens, 128 v-cols]) via PE transposes of the vT chunks.
  - Attention: for each (batch, q-chunk of 512 queries, k-tile of 128):
      S^T = K_h @ Q_h.T from kT/qT, exp on ACT, causal mask via in-place
      affine_select on the 128-col diagonal band only, then out^T
      accumulated as V'.T @ P^T where V' = [V | ones]: row 64 of the PSUM
      accumulator is the softmax denominator.
  - Projection: partial out^T = w_proj_slice.T applied per 128-row slice;
      per-core partial [1024, 4096] outputs are summed on the host.

v2 scheduling (what changed vs the 233 us baseline):
  - Clock-model-driven interleaved emission: the per-engine queues are
    in-order, so a coarse phase order (qkv0, attn0, qkv1, attn1) left the
    PE starved during attn's exp-bound stretches (HAM clock-gated to
    1.2 GHz for ~38 us) while qkv(1)/proj work sat behind it in program
    order. Emission now tracks estimated per-engine clocks and pumps
    filler quanta (qkv nt-groups, proj ob-slices) into predicted PE
    bubbles (round-robin so the out-DMA stream flows all kernel long),
    and each attention chunk pre-spreads the NEXT chunk's qkv quanta
    across its k-tiles instead of bulk-emitting them at the boundary.
  - Triangular truncation: on diagonal k-tiles only columns >= c0 (the
    un-masked range) are computed by S/exp/PV, and the causal mask is an
    in-place affine_select on just the [128,128] boundary band (base=0
    uniform across tiles) instead of a full [128,512] copy per head.
  - Softmax normalize without the DRAM bounce: the two f32 denominator
    rows are copied to partitions 0/32 of a [33,512] tile (legal
    partition starts) and broadcast EXACTLY with one full-fp32 K=33
    ones-matmul, then reciprocal_approx_fast (~18-bit, 5x faster than
    reciprocal) and one GPSIMD tensor_tensor multiply into aT. An
    earlier bf16-denominator variant failed: a per-query-scale error is
    NOT metric-invisible at early tokens, where attention averages few
    v's so per-head outputs are large while head contributions cancel
    in the final projection.
  - attn(1) q-chunk order 1,2,3,0: the tail chunk has only 4 k-tiles,
    and dummy id-matmuls sprinkled through the latency-bound tail hold
    the HAM clock gate at 8/8 so its sparse real matmuls run at 2.4 GHz.
  - x chunk 0 is DMA'd in four kt-pair pieces (QKV accumulation starts
    when piece 0 lands); the 128x128 identity for PE transposes is built
    on-chip (memset + diagonal affine_select) instead of waiting a DMA.
  - proj drains alternate ACT/DVE (GPSIMD cannot read PSUM) and each
    [128,512] ob slice DMAs out individually for a smooth tail.

All matmuls run in float32r (4-byte data, reduced-precision multiply,
~1 cycle/row measured at 512-moving warm). Known HW caveats hit here:
memsets/copies must start at partition 0/32/64/96; nc.vector.reciprocal
is ~5x slower than a copy; per-run HAM phase causes +/-8 us variance.
"""

from collections import deque

import numpy as np

D_MODEL = 1024
B, T = 2, 2048
RC = 128  # per-core qkv columns per q/k/v section == per-core w_proj rows
M = B * T
N_CORES = 8

_prog_cache = {}
_last_results = None  # BassKernelResults of the most recent run (for profiling)


def build_program(Tb=T, use_vbias=False):
    from contextlib import ExitStack

    import concourse.bass as bass
    import concourse.tile as tile
    from concourse import bacc, mybir
    from concourse.tile import add_dep_helper

    f32 = mybir.dt.float32
    f32r = mybir.dt.float32r
    bf16 = mybir.dt.bfloat16
    EXP = mybir.ActivationFunctionType.Exp
    MULT = mybir.AluOpType.mult
    IS_GE = mybir.AluOpType.is_ge
    IS_EQ = mybir.AluOpType.is_equal

    mc_per_b = Tb // 512  # x/m chunks of 512 tokens per batch
    mt_per_b = Tb // 128  # v tiles of 128 tokens per batch
    n_qc = Tb // 512      # query chunks per batch

    nc = bacc.Bacc("TRN2", target_bir_lowering=False, debug=False)
    xq = nc.dram_tensor("xq", [B * Tb // 512, 128, 8, 512], f32r,
                        kind="ExternalInput").ap()
    w3 = nc.dram_tensor("w3", [128, 8, 3 * RC], f32r, kind="ExternalInput").ap()
    wp = nc.dram_tensor("wp", [RC, D_MODEL], f32r, kind="ExternalInput").ap()
    bqk = nc.dram_tensor("bqk", [RC, 2], f32, kind="ExternalInput").ap()
    bv = None
    if use_vbias:
        bv = nc.dram_tensor("bv", [RC, 1], f32, kind="ExternalInput").ap()
    out_d = nc.dram_tensor("out", [D_MODEL, B * Tb], f32, kind="ExternalOutput").ap()

    out_r = out_d.rearrange("(nt p) m -> p nt m", p=128)  # [128, 8, Mb]

    # --- emission clock model (ns); only shapes queue order, sync is real.
    # pe starts at ~the real time the first QKV matmul can issue (x0 half0
    # landed) so the x_arrival gates compare on the same scale.
    CLK = {"pe": 9000.0, "act": 9000.0, "gps": 9000.0, "dve": 9000.0}
    import os as _os
    sched_log = [] if _os.environ.get("SCHED_DEBUG") else None

    def slog(kind):
        if sched_log is not None:
            sched_log.append((kind, CLK["pe"], CLK["act"], CLK["gps"]))

    def mm_ns(mov):
        # f32r: 1 cyc/row at >=256 moving, 4 cyc/row below; ~20ns issue
        return max(213.0, mov * 0.417) + 20.0

    def exp_ns(cols):
        return 2 * 128 * cols / 117.8 + 60.0

    with tile.TileContext(nc) as tc:
        with ExitStack() as ctx:
            singles = ctx.enter_context(tc.tile_pool(name="singles", bufs=1))
            xpool = ctx.enter_context(tc.tile_pool(name="xpool", bufs=3))
            ptp = ctx.enter_context(tc.tile_pool(name="ptp", bufs=3))
            vtp = ctx.enter_context(tc.tile_pool(name="vtp", bufs=2))
            pvcp = ctx.enter_context(tc.tile_pool(name="pvcp", bufs=2))
            rbbp = ctx.enter_context(tc.tile_pool(name="rbbp", bufs=2))
            obp = ctx.enter_context(tc.tile_pool(name="obp", bufs=8))
            ps_a = ctx.enter_context(tc.tile_pool(name="ps_a", bufs=2, space="PSUM"))
            ps_s = ctx.enter_context(tc.tile_pool(name="ps_s", bufs=2, space="PSUM"))
            ps_pv = ctx.enter_context(tc.tile_pool(name="ps_pv", bufs=2, space="PSUM"))

            # identity built on-chip (memset ones + diagonal affine_select)
            # so PE warmup starts immediately instead of waiting a DMA;
            # warmup matmuls release the HAM clock gate by the time the
            # first x piece lands
            id_sb = singles.tile([128, 128], f32r, tag="ident")
            nc.vector.memset(id_sb.bitcast(f32), 1.0)
            nc.gpsimd.affine_select(
                id_sb, id_sb, pattern=[[1, 128]], compare_op=IS_EQ,
                fill=0.0, base=0, channel_multiplier=-1,
            )
            # ones33/dn33: exact f32 denominator broadcast via one full-fp32
            # matmul. The two denominator rows live on partitions 0 and 32
            # (legal partition starts); rows 1..31 stay zero from the init
            # memset so they contribute nothing to the K=33 contraction.
            ones33 = singles.tile([33, 128], f32, tag="on33")
            dn33 = singles.tile([33, 512], f32, tag="dn33")
            nc.vector.memset(ones33, 0.0)
            nc.vector.memset(dn33, 0.0)
            nc.vector.memset(ones33[0:1, 0:64], 1.0)
            nc.vector.memset(ones33[32:33, 64:128], 1.0)
            wu_ps = ps_a.tile([128, 512], f32, tag="mm")
            for _ in range(28):
                nc.tensor.matmul(wu_ps[:, 0:128], id_sb, id_sb,
                                 start=True, stop=True)

            def warm(n):
                # dummy matmuls sprinkled through the latency-bound tail:
                # they hold the HAM clock gate at 8/8 so the sparse real
                # matmuls there run at 2.4 GHz instead of 1.2
                ds = ps_s.tile([128, 2, 512], f32, tag="s")
                for _ in range(n):
                    nc.tensor.matmul(ds[:, 0, 0:128], id_sb, id_sb,
                                     start=True, stop=True)

            # x chunks prefetched on the SP HWDGE ring; weights go through
            # the ACT HWDGE ring so the two streams don't serialize.
            # Chunk 0 is split into four kt-pair pieces (contiguous 4 KiB
            # runs per partition) chained in order: the first QKV matmuls
            # only need their own kt slice, so accumulation starts as soon
            # as piece 0 lands instead of after the whole chunk.
            x_tiles, x_dmas = [], []
            x_arrival = []
            for mc in range(B * mc_per_b):
                x_sb = xpool.tile([128, 8, 512], f32r, tag="x")
                if mc == 0:
                    prev = None
                    for p in range(4):
                        xd = nc.sync.dma_start(x_sb[:, 2 * p:2 * p + 2, :],
                                               xq[0][:, 2 * p:2 * p + 2, :])
                        if prev is not None:
                            add_dep_helper(xd.ins, prev.ins,
                                           reason="x0 piece pacing")
                        prev = xd
                elif mc == 1:
                    xd = nc.sync.dma_start(x_sb, xq[mc])
                    # chunk 0 gets the full HBM bandwidth: QKV can't start
                    # without it, so its arrival time is the kernel prologue
                    add_dep_helper(xd.ins, x_dmas[0].ins,
                                   reason="x DMA pacing")
                else:
                    xd = nc.sync.dma_start(x_sb, xq[mc])
                    add_dep_helper(xd.ins, x_dmas[mc - 2].ins,
                                   reason="x DMA pacing")
                x_tiles.append(x_sb)
                x_dmas.append(xd)
                x_arrival.append(11000.0 + 4300.0 * mc)

            w3_sb = singles.tile([128, 8, 3 * RC], f32r, tag="w3")
            nc.scalar.dma_start(w3_sb, w3)
            bqk_sb = singles.tile([RC, 2], f32, tag="bqk")
            nc.scalar.dma_start(bqk_sb, bqk)
            wp_sb = singles.tile([128, D_MODEL], f32r, tag="wp")
            nc.scalar.dma_start(wp_sb, wp)
            bv_sb = None
            if use_vbias:
                bv_sb = singles.tile([RC, 1], f32, tag="bv")
                nc.scalar.dma_start(bv_sb, bv)

            qT, kT, vb, aT = {}, {}, {}, {}
            for b in range(B):
                qT[b] = singles.tile([128, Tb], f32r, tag=f"qT{b}", name=f"qT{b}")
                kT[b] = singles.tile([128, Tb], f32r, tag=f"kT{b}", name=f"kT{b}")
                vb[b] = singles.tile([128, mt_per_b, 130], f32r, tag=f"vb{b}",
                                     name=f"vb{b}")
                aT[b] = singles.tile([128, Tb], f32r, tag=f"aT{b}", name=f"aT{b}")
                # ones columns for the softmax-denominator rows of PV
                nc.vector.memset(vb[b][:, :, 64:65].bitcast(f32), 1.0)
                nc.vector.memset(vb[b][:, :, 129:130].bitcast(f32), 1.0)

            # ---------------- quantum emitters ----------------

            def emit_qkv_group(b, mci, nt, mlo, mhi):
                """One accumulation group: qkv columns of section nt for
                tokens [mlo, mhi) of chunk mci (plus its drain op)."""
                mc = b * mc_per_b + mci
                x_sb = x_tiles[mc]
                mv = mhi - mlo
                ps = ps_a.tile([128, mv], f32, tag="mm")
                for kt in range(8):
                    nc.tensor.matmul(
                        ps,
                        w3_sb[:, kt, nt * RC:(nt + 1) * RC],
                        x_sb[:, kt, mlo:mhi],
                        start=(kt == 0), stop=(kt == 7),
                    )
                CLK["pe"] += 8 * mm_ns(mv)
                if nt < 2:
                    dest = qT[b] if nt == 0 else kT[b]
                    nc.vector.tensor_scalar_add(
                        dest[:, mci * 512 + mlo:mci * 512 + mhi], ps,
                        bqk_sb[:, nt:nt + 1],
                    )
                    CLK["dve"] += 750.0 * mv / 512
                slog(f"qkv{b}m{mci}n{nt}")
                if False:
                    pass
                else:
                    vTs = qkv_state[b]["vTs"]
                    if vTs is None:
                        vTs = vtp.tile([128, 512], f32r, tag="vT")
                        qkv_state[b]["vTs"] = vTs
                    nc.vector.tensor_copy(vTs[:, mlo:mhi], ps)
                    CLK["dve"] += 690.0 * mv / 512

            def emit_qkv_vtrans(b, mci):
                """Transpose vT chunks into natural [tokens, vcol] layout."""
                vTs = qkv_state[b]["vTs"]
                qkv_state[b]["vTs"] = None
                tp = ps_a.tile([128, 512], f32, tag="mm")
                for ms in range(4):
                    nc.tensor.transpose(
                        tp[:, ms * 128:(ms + 1) * 128].bitcast(f32r),
                        vTs[:, ms * 128:(ms + 1) * 128],
                        id_sb,
                    )
                CLK["pe"] += 4 * 160.0
                for ms in range(4):
                    mt = mci * 4 + ms
                    sl = tp[:, ms * 128:(ms + 1) * 128].bitcast(f32r)
                    # single strided copy covering V cols {0:64} u {65:129}
                    dst = vb[b][:, mt, :].rearrange("p (two f) -> p two f",
                                                    f=65)[:, :, 0:64]
                    src = sl.rearrange("p (two f) -> p two f", f=64)
                    nc.vector.tensor_copy(dst, src)
                    CLK["dve"] += 340.0
                slog(f"qkv{b}m{mci}T")

            qkv_state = {b: {"vTs": None} for b in range(B)}

            proj_pending = deque()  # (avail_ns, b, q_lo, qw, np_)
            ob_alt = [0]

            def emit_proj_np(b, q_lo, qw, np_):
                for j in range(2):
                    nt = np_ * 2 + j
                    ps_t = ps_a.tile([128, 512], f32, tag="mm")
                    ps = ps_t[:, 0:qw]
                    nc.tensor.matmul(
                        ps,
                        wp_sb[:, nt * 128:(nt + 1) * 128],
                        aT[b][:, q_lo:q_lo + qw],
                        start=True, stop=True,
                    )
                    CLK["pe"] += mm_ns(qw)
                    ob_t = obp.tile([128, 512], f32, tag="ob")
                    ob = ob_t[:, 0:qw]
                    # GPSIMD cannot read PSUM; alternate the drain between
                    # the ACT and DVE engines to balance their queues
                    if ob_alt[0] % 2 == 0:
                        nc.scalar.copy(ob, ps)
                        CLK["act"] += 580.0 * qw / 512
                    else:
                        nc.vector.tensor_copy(ob, ps)
                        CLK["dve"] += 690.0 * qw / 512
                    ob_alt[0] += 1
                    slog(f"proj{b}q{q_lo // 512}n{np_}")
                    nc.sync.dma_start(
                        out_r[:, nt, b * Tb + q_lo: b * Tb + q_lo + qw],
                        ob,
                    )

            class QkvStream:
                def __init__(self, b):
                    self.b = b
                    self.items = []
                    for mci in range(mc_per_b):
                        mc = b * mc_per_b + mci
                        for nt in range(3):
                            self.items.append(
                                ("g", mci, nt, 0, 512, x_arrival[mc]))
                        self.items.append(("t", mci, 0, 0, 0, x_arrival[mc]))
                    self.pos = 0

                def done_through(self, mci):
                    """True if all quanta of chunks <= mci are emitted."""
                    for it in self.items[self.pos:]:
                        if it[1] <= mci:
                            return False
                    return True

                def avail(self):
                    if self.pos >= len(self.items):
                        return None
                    return self.items[self.pos][5]

                def step(self):
                    it = self.items[self.pos]
                    self.pos += 1
                    if it[0] == "g":
                        emit_qkv_group(self.b, it[1], it[2], it[3], it[4])
                    else:
                        emit_qkv_vtrans(self.b, it[1])

            qkv_streams = {b: QkvStream(b) for b in range(B)}

            pump_alt = [0]

            def _pump_qkv():
                for b in range(B):
                    qs = qkv_streams[b]
                    av = qs.avail()
                    if av is not None and av <= CLK["pe"] + 1500.0:
                        qs.step()
                        return True
                return False

            def _pump_proj():
                if not proj_pending:
                    return False
                avail, b, q_lo, qw, np_ = proj_pending[0]
                if avail > CLK["pe"] + 800.0:
                    return False
                proj_pending.popleft()
                emit_proj_np(b, q_lo, qw, np_)
                return True

            def pump(target):
                """Emit filler quanta until the estimated PE clock reaches
                target (or nothing is ready). Round-robin qkv/proj so the
                output DMA stream flows throughout instead of piling up at
                the tail."""
                while CLK["pe"] < target:
                    pump_alt[0] ^= 1
                    first, second = ((_pump_qkv, _pump_proj)
                                     if pump_alt[0] else
                                     (_pump_proj, _pump_qkv))
                    if not (first() or second()):
                        break

            def ensure_qkv(b, mci):
                qs = qkv_streams[b]
                while not qs.done_through(mci):
                    qs.step()

            # ---------------- attention ----------------

            def emit_attn_chunk(b, q_lo, qw, next_need=None, proj_keep=3,
                                tail_warm=False):
                """Attention for queries [q_lo, q_lo+qw) of batch b.
                next_need=(b', mci'): the qkv chunk the NEXT attention
                chunk requires; its quanta are spread across this chunk's
                k-tiles instead of bulk-emitted at the boundary. proj_keep:
                force-drain proj backlog beyond this depth (1 quantum/kt)
                so the out-DMA stream flows instead of piling at the tail."""
                nkt = (q_lo + qw) // 128
                kt_diag = q_lo // 128
                pv0_t = ps_pv.tile([65, 512], f32, tag="pv", name="pv0")
                pv1_t = ps_pv.tile([65, 512], f32, tag="pv", name="pv1")
                pvs = (pv0_t[:, 0:qw], pv1_t[:, 0:qw])
                pend = None  # (kt, c0, psrc, p_done_est)

                def spread(kt):
                    if next_need is not None:
                        qs = qkv_streams[next_need[0]]
                        if (not qs.done_through(next_need[1])
                                and kt % 3 == 2):
                            qs.step()
                            return
                    if len(proj_pending) > proj_keep:
                        avail, pb, pq, pw, pnp = proj_pending[0]
                        if avail <= CLK["pe"] + 800.0:
                            proj_pending.popleft()
                            emit_proj_np(pb, pq, pw, pnp)

                def emit_pv(kt, c0, psrc, p_done):
                    pump(p_done - 500.0)
                    for h in (0, 1):
                        nc.tensor.matmul(
                            pvs[h][:, c0:qw],
                            vb[b][:, kt, h * 65:(h + 1) * 65],
                            psrc[:, h, c0:qw],
                            start=(kt == 0), stop=(kt == nkt - 1),
                        )
                    CLK["pe"] = max(CLK["pe"], p_done) + 2 * mm_ns(qw - c0)
                    slog(f"pv{b}q{q_lo // 512}k{kt}")
                    if tail_warm and kt >= nkt - 6:
                        warm(2)

                for kt in range(nkt):
                    c0 = max(0, kt * 128 - q_lo)
                    s_t = ps_s.tile([128, 2, 512], f32, tag="s")
                    s = s_t[:, :, 0:qw]
                    for h in (0, 1):
                        nc.tensor.matmul(
                            s[:, h, c0:qw],
                            kT[b][h * 64:(h + 1) * 64,
                                  kt * 128:(kt + 1) * 128],
                            qT[b][h * 64:(h + 1) * 64,
                                  q_lo + c0:q_lo + qw],
                            start=True, stop=True,
                        )
                    CLK["pe"] += 2 * mm_ns(qw - c0)
                    s_done = CLK["pe"]
                    pt_t = ptp.tile([128, 2, 512], f32r, tag="pt")
                    pt = pt_t[:, :, 0:qw]
                    nc.scalar.activation(pt[:, :, c0:qw], s[:, :, c0:qw],
                                         EXP, scale=0.125)
                    CLK["act"] = max(CLK["act"], s_done + 150.0) \
                        + exp_ns(qw - c0)
                    p_done = CLK["act"]
                    if kt >= kt_diag:  # diagonal k-tile: mask the 128-col band
                        for h in (0, 1):
                            nc.gpsimd.affine_select(
                                pt[:, h, c0:c0 + 128],
                                pt[:, h, c0:c0 + 128],
                                pattern=[[1, 128]],
                                compare_op=IS_GE,
                                fill=0.0,
                                base=0,
                                channel_multiplier=-1,
                            )
                        CLK["gps"] = max(CLK["gps"], p_done) + 2 * 250.0
                        p_done = CLK["gps"]
                    if pend is not None:
                        emit_pv(*pend)
                        spread(kt)
                    pend = (kt, c0, pt, p_done)
                emit_pv(*pend)

                # normalize: denom rows -> f32 ones-matmul partition
                # broadcast -> fast reciprocal -> one fused multiply into aT
                d0 = q_lo % 512
                pvc_t = pvcp.tile([128, 512], f32, tag="pvc")
                pvc = pvc_t[:, 0:qw]
                nc.vector.tensor_copy(pvc[0:64], pvs[0][0:64])
                nc.vector.tensor_copy(pvc[64:128], pvs[1][0:64])
                nc.vector.tensor_copy(dn33[0:1, d0:d0 + qw], pvs[0][64:65])
                nc.vector.tensor_copy(dn33[32:33, d0:d0 + qw], pvs[1][64:65])
                CLK["dve"] = max(CLK["dve"], CLK["pe"]) \
                    + (2 * 350.0 + 2 * 270.0) * qw / 512
                pump(CLK["dve"] - 200.0)  # bcast matmul waits the dn copies
                ps_rb_t = ps_a.tile([128, 512], f32, tag="mm")
                ps_rb = ps_rb_t[:, 0:qw]
                nc.tensor.matmul(ps_rb, ones33, dn33[:, d0:d0 + qw],
                                 start=True, stop=True)
                CLK["pe"] = max(CLK["pe"], CLK["dve"]) + qw * 1.67 + 50.0
                rbb_t = rbbp.tile([128, 512], f32, tag="rbb")
                rbb = rbb_t[:, 0:qw]
                nc.vector.reciprocal_approx_fast(rbb, ps_rb)
                CLK["dve"] = max(CLK["dve"], CLK["pe"]) + 700.0 * qw / 512
                dst = aT[b][:, q_lo:q_lo + qw]
                nc.gpsimd.tensor_tensor(dst, pvc, rbb, op=MULT)
                CLK["gps"] = max(CLK["gps"], CLK["dve"]) + 700.0 * qw / 512
                slog(f"norm{b}q{q_lo // 512}")
                if tail_warm:
                    warm(3)
                if use_vbias:
                    nc.vector.tensor_scalar_add(dst, dst, bv_sb)
                avail = CLK["gps"]
                for np_ in range(4):
                    proj_pending.append((avail, b, q_lo, qw, np_))

            # ---------------- top-level schedule ----------------

            # attention chunk order: b0 ascending, then b1 chunks 1,2,3
            # and finally b1 queries [256,512) then [0,256) — the very last
            # chunk has only 2 k-tiles so the serial end-of-kernel chain
            # (attn + normalize + proj + out DMA) is minimal. next_need
            # pre-spreads the qkv chunk the following attention chunk needs.
            order = [(0, 0, 512), (0, 512, 512), (0, 1024, 512),
                     (0, 1536, 512),
                     (1, 512, 512), (1, 1024, 512), (1, 1536, 512),
                     (1, 0, 512)]
            for i, (b, q_lo, qw) in enumerate(order):
                ensure_qkv(b, (q_lo + qw - 1) // 512)
                if i + 1 < len(order):
                    nb, nq, nw = order[i + 1]
                    nxt = (nb, (nq + nw - 1) // 512)
                else:
                    nxt = None
                tail = i >= len(order) - 2
                emit_attn_chunk(b, q_lo, qw, next_need=nxt,
                                proj_keep=0 if tail else 3, tail_warm=tail)
            # drain leftovers
            ensure_qkv(1, mc_per_b - 1)
            while proj_pending:
                _, b, q_lo, qw, np_ = proj_pending.popleft()
                emit_proj_np(b, q_lo, qw, np_)
                warm(1)

    if sched_log is not None:
        print("=== emission schedule (kind, pe_us) ===")
        line = []
        for kind, pe, act, gps in sched_log:
            line.append(f"{kind}@{pe/1000:.0f}")
            if len(line) == 10:
                print("  " + " ".join(line)); line = []
        if line:
            print("  " + " ".join(line))
        print(f"final CLK: pe={CLK['pe']/1000:.1f} act={CLK['act']/1000:.1f} "
              f"gps={CLK['gps']/1000:.1f} dve={CLK['dve']/1000:.1f}")
    nc.compile()
    return nc


def make_in_maps(x, w_qkv, b_qkv, use_vbias):
    """Host-side shard prep. Returns per-core input maps (w_proj added later)."""
    Mx = x.shape[0] * x.shape[1]
    # [chunks, 128p, 8kt, 512m]: per-partition-contiguous 16 KiB blocks so
    # each chunk DMA uses 128 big descriptors instead of 1024 2-KiB ones
    xq = np.ascontiguousarray(
        x.reshape(Mx // 512, 512, 8, 128).transpose(0, 3, 2, 1)
    )
    in_maps = []
    for c in range(N_CORES):
        w3c = np.concatenate(
            [w_qkv[:, s * D_MODEL + c * RC: s * D_MODEL + (c + 1) * RC]
             for s in range(3)],
            axis=1,
        )
        # [128p, 8kt, 384]: per-partition-contiguous blocks for big DMA
        # descriptors
        w3c = np.ascontiguousarray(
            w3c.reshape(8, 128, 3 * RC).transpose(1, 0, 2)
        )
        bqkc = np.ascontiguousarray(
            np.stack(
                [b_qkv[c * RC:(c + 1) * RC],
                 b_qkv[D_MODEL + c * RC: D_MODEL + (c + 1) * RC]],
                axis=1,
            )
        )
        im = {"xq": xq, "w3": w3c, "bqk": bqkc,
              "ident": np.eye(128, dtype=np.float32)}
        if use_vbias:
            im["bv"] = np.ascontiguousarray(
                b_qkv[2 * D_MODEL + c * RC: 2 * D_MODEL + (c + 1) * RC][:, None]
            )
        in_maps.append(im)
    return in_maps


def kernel(x, w_qkv, b_qkv, w_proj, b_proj):
    from concourse.bass_utils import run_bass_kernel_spmd

    x = np.asarray(x, dtype=np.float32)
    w_qkv = np.asarray(w_qkv, dtype=np.float32)
    b_qkv = np.asarray(b_qkv, dtype=np.float32)
    w_proj = np.asarray(w_proj, dtype=np.float32)
    b_proj = np.asarray(b_proj, dtype=np.float32)

    use_vbias = bool(np.any(b_qkv[2 * D_MODEL:]))
    key = (T, use_vbias)
    if key not in _prog_cache:
        _prog_cache[key] = build_program(T, use_vbias)
    nc = _prog_cache[key]

    in_maps = make_in_maps(x, w_qkv, b_qkv, use_vbias)
    for c in range(N_CORES):
        in_maps[c]["wp"] = np.ascontiguousarray(w_proj[c * RC:(c + 1) * RC, :])

    res = run_bass_kernel_spmd(nc, in_maps, core_ids=list(range(N_CORES)))
    global _last_results
    _last_results = res
    total = res.results[0]["out"].copy()
    for c in range(1, N_CORES):
        total += res.results[c]["out"]
    out = total.T.reshape(B, T, D_MODEL) + b_proj[None, None, :]
    return np.ascontiguousarray(out.astype(np.float32))
